# revision 1
# baseline (speedup 1.0000x reference)
"""Trainium2 Bass kernel for DRModel: ragged basket-pool + masked GRU.

Computation (matches the jax reference):
  pooled[b,s,:] = mean over valid k of encode[items[b,s,k]]   (basket pooling)
  GRU over s with packed-sequence masking:
    h' = where(s < len[b], GRUCell(pooled[b,s], h), h)
    y[b,s] = where(s < len[b], h', 0)

Sharding: data-parallel over batch, 32 users per core, 8 cores.
Embedding table + GRU weights replicated.

Device strategy per core:
  * Pooling: indirect-DMA gather 120 embedding rows (6 baskets x 20 items)
    into SBUF [tokens, D], then PE matmul with a host-built [tokens, 6]
    block weight matrix (mask & 1/len folded in) -> psum [D, baskets].
    Basket order is s-major (col = s*32 + b) so GRU slices are contiguous.
  * gx = W_ih @ x precomputed in 480-col chunks (chunk = 15 GRU steps);
    biases folded in; z-gate weights/bias negated so sigmoid gives (1-z).
  * GRU: per step 3 matmuls [128,128]x[128,32] (r,z,n) + fused elementwise:
      a_rz = psum_rz + gx_rz ; sig = sigmoid(a_rz)
      ghn  = psum_n + b_hhn  (scalar-engine Identity w/ bias)
      n    = tanh(sig_r * ghn + gx_n)
      h'   = h + mask * (sig_zc * (n - h))
      y_s  = mstrip_s * transpose(h')   (PE transpose, [32,128])
"""

import numpy as np

B, S, K, D, V = 256, 50, 20, 128, 100000
NCORES = 8
BL = B // NCORES          # 32 users per core
NB = BL * S               # 1600 baskets per core
BPT = 6                   # baskets per token-tile
TPT = BPT * K             # 120 tokens per tile
NT = (NB + BPT - 1) // BPT            # 267 tiles
G = 32                    # tiles per DMA group
NG = (NT + G - 1) // G                # 9 groups
NT_PAD = NG * G                       # 288
NBC_PAD = NT * BPT                    # 1602 pooled cols

# chunking: 80 tiles = 480 cols = 15 steps per chunk (last chunk ragged)
CH_TILES = 80
CH_COLS = CH_TILES * BPT              # 480
CH_STEPS = CH_COLS // BL              # 15
CHUNKS = []  # (tile0, ntiles, col0, ncols_used, step0, nsteps)
t0 = 0
while t0 < NT:
    nt = min(CH_TILES, NT - t0)
    col0 = t0 * BPT
    ncols_pad = nt * BPT
    s0 = col0 // BL
    nsteps = min(S - s0, CH_STEPS)
    CHUNKS.append((t0, nt, col0, ncols_pad, s0, nsteps))
    t0 += nt

_CACHE = {}


def _build():
    if "nc" in _CACHE:
        return _CACHE["nc"]
    import concourse.bacc as bacc
    import concourse.mybir as mybir
    import concourse.tile as tile
    from concourse import bass
    from concourse.masks import make_identity

    f32 = mybir.dt.float32
    i32 = mybir.dt.int32
    AF = mybir.ActivationFunctionType

    nc = bacc.Bacc("TRN2", target_bir_lowering=False, debug=False,
                   num_devices=NCORES)

    table = nc.dram_tensor("table", [V, D], f32, kind="ExternalInput")
    idx_d = nc.dram_tensor("idx", [NG, 128, G], i32, kind="ExternalInput")
    wmat_d = nc.dram_tensor("wmat", [NG, 128, G * BPT], f32, kind="ExternalInput")
    wihT_d = nc.dram_tensor("wihT", [3, D, D], f32, kind="ExternalInput")
    whhT_d = nc.dram_tensor("whhT", [3, D, D], f32, kind="ExternalInput")
    bias_d = nc.dram_tensor("bias4", [D, 4], f32, kind="ExternalInput")
    mask_d = nc.dram_tensor("mask", [D, NB], f32, kind="ExternalInput")
    mstrip_d = nc.dram_tensor("mstrip", [BL, S], f32, kind="ExternalInput")
    h0_d = nc.dram_tensor("h0T", [D, BL], f32, kind="ExternalInput")
    y_d = nc.dram_tensor("y", [BL, S * D], f32, kind="ExternalOutput")
    hout_d = nc.dram_tensor("hout", [BL, D], f32, kind="ExternalOutput")

    with tile.TileContext(nc) as tc:
        with (
            tc.tile_pool(name="const", bufs=1) as cp,
            tc.tile_pool(name="big", bufs=1) as bigp,
            tc.tile_pool(name="emb", bufs=4) as ep,
            tc.tile_pool(name="grp", bufs=2) as gp,
            tc.tile_pool(name="gru", bufs=3) as grp,
            tc.tile_pool(name="hh", bufs=2) as hp,
            tc.tile_pool(name="ppool", bufs=2, space="PSUM") as ppp,
            tc.tile_pool(name="pgx", bufs=2, space="PSUM") as pgx,
            tc.tile_pool(name="pgh", bufs=2, space="PSUM") as pgh,
            tc.tile_pool(name="ptr", bufs=2, space="PSUM") as ptr,
        ):
            # ---- constants ----
            wih_sb = cp.tile([D, 3 * D], f32, tag="wih")
            whh_sb = cp.tile([D, 3 * D], f32, tag="whh")
            for g in range(3):
                nc.sync.dma_start(out=wih_sb[:, g * D:(g + 1) * D], in_=wihT_d[g])
                nc.sync.dma_start(out=whh_sb[:, g * D:(g + 1) * D], in_=whhT_d[g])
            bias_sb = cp.tile([D, 4], f32, tag="bias")
            nc.sync.dma_start(out=bias_sb[:], in_=bias_d[:])
            mask_sb = cp.tile([D, NB], f32, tag="mask")
            nc.sync.dma_start(out=mask_sb[:], in_=mask_d[:])
            mstrip_sb = cp.tile([BL, S], f32, tag="mstrip")
            nc.sync.dma_start(out=mstrip_sb[:], in_=mstrip_d[:])
            ident = cp.tile([128, 128], f32, tag="ident")
            make_identity(nc, ident[:])
            h_first = hp.tile([D, BL], f32, tag="h")
            nc.sync.dma_start(out=h_first[:], in_=h0_d[:])

            y_sb = bigp.tile([BL, S * D], f32, tag="y")

            # ---- pooling + gx, chunk by chunk ----
            pooled_c = []
            gxrz_c = []
            gxn_c = []
            for (ct0, cnt, col0, ncp, s0, nst) in CHUNKS:
                pp = ppp.tile([D, ncp], f32, tag="pp")
                for j in range(cnt):
                    t = ct0 + j
                    g, jg = divmod(t, G)
                    if jg == 0:
                        ig = gp.tile([128, G], i32, tag="ig")
                        nc.sync.dma_start(out=ig[:], in_=idx_d[g])
                        wg = gp.tile([128, G * BPT], f32, tag="wg")
                        nc.sync.dma_start(out=wg[:], in_=wmat_d[g])
                    emb = ep.tile([TPT, D], f32, tag="emb")
                    nc.gpsimd.indirect_dma_start(
                        out=emb[:],
                        out_offset=None,
                        in_=table[:],
                        in_offset=bass.IndirectOffsetOnAxis(
                            ap=ig[:TPT, jg:jg + 1], axis=0),
                    )
                    nc.tensor.matmul(
                        out=pp[:, j * BPT:(j + 1) * BPT],
                        lhsT=emb[:],
                        rhs=wg[:TPT, jg * BPT:(jg + 1) * BPT],
                        start=True, stop=True,
                    )
                pool_sb = bigp.tile([D, ncp], f32, tag=f"pool{ct0}")
                nc.vector.tensor_copy(pool_sb[:], pp[:])
                pooled_c.append(pool_sb)

                ncols = nst * BL
                gxrz = bigp.tile([D, nst * 2 * BL], f32, tag=f"gxrz{ct0}")
                gxn = bigp.tile([D, ncols], f32, tag=f"gxn{ct0}")
                gxrz_v = gxrz[:].rearrange("p (s h b) -> p s h b", h=2, b=BL)
                for g in range(3):
                    px = pgx.tile([D, ncols], f32, tag="px")
                    nc.tensor.matmul(
                        out=px[:],
                        lhsT=wih_sb[:, g * D:(g + 1) * D],
                        rhs=pool_sb[:, :ncols],
                        start=True, stop=True,
                    )
                    if g < 2:
                        dst = gxrz_v[:, :, g, :]
                    else:
                        dst = gxn[:]
                    nc.vector.tensor_scalar_add(dst, px[:], bias_sb[:, g:g + 1])
                gxrz_c.append(gxrz)
                gxn_c.append(gxn)

            # ---- GRU ----
            h_prev = h_first
            for s in range(S):
                ci = min(s // CH_STEPS, len(CHUNKS) - 1)
                sl = s - CHUNKS[ci][4]
                gxrz = gxrz_c[ci]
                gxn = gxn_c[ci]

                pg = pgh.tile([D, 3 * BL], f32, tag="pg")
                for g in range(3):
                    nc.tensor.matmul(
                        out=pg[:, g * BL:(g + 1) * BL],
                        lhsT=whh_sb[:, g * D:(g + 1) * D],
                        rhs=h_prev[:],
                        start=True, stop=True,
                    )
                a_rz = grp.tile([D, 2 * BL], f32, tag="a_rz")
                nc.vector.tensor_add(a_rz[:], pg[:, 0:2 * BL],
                                     gxrz[:, sl * 2 * BL:(sl + 1) * 2 * BL])
                sig = grp.tile([D, 2 * BL], f32, tag="sig")
                nc.scalar.activation(sig[:], a_rz[:], AF.Sigmoid)
                ghn = grp.tile([D, BL], f32, tag="ghn")
                nc.scalar.activation(ghn[:], pg[:, 2 * BL:3 * BL], AF.Identity,
                                     bias=bias_sb[:, 3:4])
                rn = grp.tile([D, BL], f32, tag="rn")
                nc.vector.tensor_mul(rn[:], sig[:, 0:BL], ghn[:])
                npre = grp.tile([D, BL], f32, tag="npre")
                nc.vector.tensor_add(npre[:], rn[:],
                                     gxn[:, sl * BL:(sl + 1) * BL])
                nt_ = grp.tile([D, BL], f32, tag="nt")
                nc.scalar.activation(nt_[:], npre[:], AF.Tanh)
                u = grp.tile([D, BL], f32, tag="u")
                nc.vector.tensor_sub(u[:], nt_[:], h_prev[:])
                v = grp.tile([D, BL], f32, tag="v")
                nc.vector.tensor_mul(v[:], sig[:, BL:2 * BL], u[:])
                mv = grp.tile([D, BL], f32, tag="mv")
                nc.vector.tensor_mul(mv[:], v[:],
                                     mask_sb[:, s * BL:(s + 1) * BL])
                h_next = hp.tile([D, BL], f32, tag="h")
                nc.vector.tensor_add(h_next[:], h_prev[:], mv[:])

                tr = ptr.tile([BL, D], f32, tag="tr")
                nc.tensor.transpose(out=tr[:], in_=h_next[:], identity=ident[:])
                nc.vector.tensor_scalar_mul(y_sb[:, s * D:(s + 1) * D], tr[:],
                                            mstrip_sb[:, s:s + 1])
                h_prev = h_next

            hout_sb = cp.tile([BL, D], f32, tag="hout")
            tr = ptr.tile([BL, D], f32, tag="tr")
            nc.tensor.transpose(out=tr[:], in_=h_prev[:], identity=ident[:])
            nc.vector.tensor_copy(hout_sb[:], tr[:])

            nc.sync.dma_start(out=y_d[:], in_=y_sb[:])
            nc.sync.dma_start(out=hout_d[:], in_=hout_sb[:])

    nc.compile()
    _CACHE["nc"] = nc
    return nc


def _host_prep(items, basket_len, lengths, encode, w_ih, w_hh, b_ih, b_hh, h0):
    """Build per-core input maps."""
    items = np.asarray(items).astype(np.int64)
    basket_len = np.asarray(basket_len).astype(np.int64)
    lengths = np.asarray(lengths).astype(np.int64)
    encode = np.ascontiguousarray(np.asarray(encode), dtype=np.float32)
    w_ih = np.asarray(w_ih, dtype=np.float32)
    w_hh = np.asarray(w_hh, dtype=np.float32)
    b_ih = np.asarray(b_ih, dtype=np.float32)
    b_hh = np.asarray(b_hh, dtype=np.float32)
    h0 = np.asarray(h0, dtype=np.float32)

    # shared weights
    wihT = np.stack([w_ih[g * D:(g + 1) * D].T.copy() for g in range(3)])
    whhT = np.stack([w_hh[g * D:(g + 1) * D].T.copy() for g in range(3)])
    wihT[1] = -wihT[1]
    whhT[1] = -whhT[1]
    bias4 = np.zeros((D, 4), np.float32)
    bias4[:, 0] = b_ih[0:D] + b_hh[0:D]
    bias4[:, 1] = -(b_ih[D:2 * D] + b_hh[D:2 * D])
    bias4[:, 2] = b_ih[2 * D:3 * D]
    bias4[:, 3] = b_hh[2 * D:3 * D]
    wihT = np.ascontiguousarray(wihT)
    whhT = np.ascontiguousarray(whhT)

    # basket weights: mask/len, [B,S,K]
    karange = np.arange(K)[None, None, :]
    wgt = (karange < basket_len[..., None]).astype(np.float32)
    wgt /= basket_len[..., None].astype(np.float32)

    in_maps = []
    for c in range(NCORES):
        bsl = slice(c * BL, (c + 1) * BL)
        # token order: basket = s*BL + b -> items[c*BL+b, s, k]
        it_c = np.transpose(items[bsl], (1, 0, 2)).reshape(NB, K)  # [1600,20]
        wg_c = np.transpose(wgt[bsl], (1, 0, 2)).reshape(NB, K)
        # pad baskets to NT_PAD*BPT
        it_pad = np.zeros((NT_PAD * BPT, K), np.int64)
        wg_pad = np.zeros((NT_PAD * BPT, K), np.float32)
        it_pad[:NB] = it_c
        wg_pad[:NB] = wg_c
        # tiles: [NT_PAD, BPT, K]
        it_t = it_pad.reshape(NT_PAD, BPT, K)
        wg_t = wg_pad.reshape(NT_PAD, BPT, K)
        # idx rows: partition p = cbpt*K + k  -> [NT_PAD, 128]
        idx_rows = np.zeros((NT_PAD, 128), np.int32)
        idx_rows[:, :TPT] = it_t.reshape(NT_PAD, TPT).astype(np.int32)
        # grouped transposed: idx[g, p, j] = idx_rows[g*G+j, p]
        idx_g = np.ascontiguousarray(
            idx_rows.reshape(NG, G, 128).transpose(0, 2, 1))
        # wmat[g, p, j*BPT + c] = wg of tile g*G+j, basket c, row p=c*K+k
        wmat = np.zeros((NG, G, 128, BPT), np.float32)
        rows = (np.arange(BPT)[:, None] * K + np.arange(K)[None, :])  # [BPT,K]
        for cb in range(BPT):
            wmat[:, :, rows[cb], cb] = wg_t[:, cb, :].reshape(NG, G, K)
        wmat = np.ascontiguousarray(
            wmat.transpose(0, 2, 1, 3).reshape(NG, 128, G * BPT))

        len_c = lengths[bsl]  # [32]
        m = (np.arange(S)[:, None] < len_c[None, :]).astype(np.float32)  # [S,BL]
        mask = np.ascontiguousarray(
            np.broadcast_to(m.reshape(1, NB), (D, NB)))
        mstrip = np.ascontiguousarray(m.T)  # [BL, S]
        h0T = np.ascontiguousarray(h0[0, bsl].T)  # [D, BL]

        in_maps.append({
            "table": encode,
            "idx": idx_g,
            "wmat": wmat,
            "wihT": wihT,
            "whhT": whhT,
            "bias4": bias4,
            "mask": mask,
            "mstrip": mstrip,
            "h0T": h0T,
        })
    return in_maps


def kernel(items, basket_len, lengths, encode, w_ih, w_hh, b_ih, b_hh, h0,
           _trace=False):
    from concourse.bass_utils import run_bass_kernel_spmd

    nc = _build()
    in_maps = _host_prep(items, basket_len, lengths, encode,
                         w_ih, w_hh, b_ih, b_hh, h0)
    res = run_bass_kernel_spmd(nc, in_maps, core_ids=list(range(NCORES)),
                               trace=_trace)
    y = np.zeros((B, S, D), np.float32)
    h_u = np.zeros((1, B, D), np.float32)
    for c in range(NCORES):
        y[c * BL:(c + 1) * BL] = res.results[c]["y"].reshape(BL, S, D)
        h_u[0, c * BL:(c + 1) * BL] = res.results[c]["hout"]
    if _trace:
        kernel._last_exec_ns = res.exec_time_ns
        kernel._last_res = res
    return y, h_u
